# revision 1
# baseline (speedup 1.0000x reference)
"""
CRFTagger NLL loss on 8 Trainium2 NeuronCores (Bass/Tile).

Strategy
--------
Data-parallel over batch: each of the 8 cores runs the CRF forward algorithm
for 16 of the 128 sequences.  The log-semiring scan runs in the *exp domain*
with a constant per-step shift s (s = log Perron-eigenvalue of exp(trans)
+ 0.5, computed on host from the transitions input), so one scan step is
just one PE matmul + one DVE elementwise multiply:

    P_{t+1} = (E^T @ P_t) * exp(feat_t)        E = exp(trans - s)  [C,C]

No per-step logsumexp / max / renormalization: with the Perron shift the
magnitudes drift only a few e-folds over the whole scan (validated:
|log P| < 10); states/weights are bf16 (fp32 exponent range, overflow-proof).

The scan is a latency-bound serial chain (~440ns per matmul+mul round trip),
so the kernel halves the chain length with a *bidirectional* split: a forward
recursion over the first half of time and an independent backward recursion
over the second half run concurrently, interleaved on the PE and DVE engines.
For variable-length sequences the backward pass is time-ALIGNED on host: the
feature stream of sequence b is reversed and shifted so every sequence "ends"
at the same device iteration, making the backward init a single shared
one-hot STOP seed.  Both chains store their full state history in SBUF and
ship it out (overlapped with compute); the host picks, per sequence, the
meeting-point pair

    logZ_b = log( sum_j Pf_{t+1}[j,b] * X_{m+1}[j,b] / exp(feats[b,t,j]) )
             + (len_b + 1) * s ,   t = max(0, len_b - 256)

The gold-path score (pure gathers over tags, O(B*T) with zero reuse) is
evaluated on host during unsharding.
"""

import sys

import ml_dtypes
import numpy as np

sys.path.insert(0, "/opt/trn_rl_repo")

import concourse.bacc as bacc  # noqa: E402
import concourse.mybir as mybir  # noqa: E402
from concourse import tile  # noqa: E402
from concourse.bass_utils import run_bass_kernel_spmd  # noqa: E402
from concourse.tile_rust import add_dep_helper  # noqa: E402

B, T, C = 128, 512, 128
N_CORES = 8
BL = B // N_CORES   # 16 sequences per core
HF = T // 2         # 256
NFW = HF + 1        # forward steps  (needs P up to slot 257 when len=512)
NBW = HF            # backward steps
CH = 64             # time-steps per feature chunk (DMA/exp granularity)

_NC = None
LAST_RESULT = None  # BassKernelResults of the most recent run (for profiling)


FIRST_CH = 16  # small first chunk: the scan can start after a tiny DMA+exp


def _chunks(n):
    out, lo = [], 0
    if n > FIRST_CH:
        out.append((0, FIRST_CH))
        lo = FIRST_CH
    while lo < n:
        out.append((lo, min(lo + CH, n)))
        lo += CH
    return out


def _build_nc():
    nc = bacc.Bacc("TRN2", target_bir_lowering=False, debug=False)
    fp32 = mybir.dt.float32
    fp16 = mybir.dt.bfloat16
    ffw_h = nc.dram_tensor("ffw", [C, NFW, BL], fp32, kind="ExternalInput")
    fbw_h = nc.dram_tensor("fbw", [C, NBW, BL], fp32, kind="ExternalInput")
    # one constant block = one DMA: [E | E^T | seedF | seedB]
    konst_h = nc.dram_tensor(
        "konst", [C, 2 * C + 2 * BL], fp16, kind="ExternalInput"
    )
    pf_h = nc.dram_tensor("pfout", [C, (NFW + 1) * BL], fp16, kind="ExternalOutput")
    xb_h = nc.dram_tensor("xbout", [C, (NBW + 1) * BL], fp16, kind="ExternalOutput")

    with tile.TileContext(nc) as tc:
        with (
            tc.tile_pool(name="consts", bufs=1) as consts,
            tc.tile_pool(name="ffw", bufs=len(_chunks(NFW))) as ffwp,
            tc.tile_pool(name="fbw", bufs=len(_chunks(NBW))) as fbwp,
            tc.tile_pool(name="hist", bufs=1) as hist,
            tc.tile_pool(name="mpsF", bufs=2, space="PSUM") as mpsF,
            tc.tile_pool(name="mpsB", bufs=2, space="PSUM") as mpsB,
        ):
            konst = consts.tile([C, 2 * C + 2 * BL], fp16)
            nc.sync.dma_start(out=konst[:], in_=konst_h[:])
            emat = konst[:, 0:C]
            ematT = konst[:, C : 2 * C]
            seedF = konst[:, 2 * C : 2 * C + BL]
            seedB = konst[:, 2 * C + BL : 2 * C + 2 * BL]

            # state histories: slot k of PF is P_k (k=0..NFW), slot m of XB
            # is X_m (m=0..NBW).  Slot 0 (the seed) lives in the konst tile
            # instead — the host never reads slot 0 of the shipped history.
            PF = hist.tile([C, (NFW + 1) * BL], fp16)
            XB = hist.tile([C, (NBW + 1) * BL], fp16)

            # stream feats in chunks, exponentiating in place
            def load_feats(pool, dram, spans):
                tiles = []
                for lo, hi in spans:
                    f = pool.tile([C, (hi - lo) * BL], fp32)
                    nc.sync.dma_start(
                        out=f[:],
                        in_=dram[:, lo:hi, :].rearrange("c t b -> c (t b)"),
                    )
                    nc.scalar.activation(
                        f[:], f[:], mybir.ActivationFunctionType.Exp
                    )
                    tiles.append(f)
                return tiles

            def slot_map(spans):
                m = {}
                for i, (lo, hi) in enumerate(spans):
                    for k in range(lo, hi):
                        m[k] = (i, k - lo)
                return m


            # first chunk of each chain is DMA'd/exp'd first so the scan can
            # start while the remaining chunks stream in
            fw_spans = _chunks(NFW)
            bw_spans = _chunks(NBW)
            # interleave F/B chunk loads so neither chain's next chunk gets
            # queued behind all of the other chain's DMAs
            ffw, fbw = [], []
            for i in range(max(len(fw_spans), len(bw_spans))):
                if i < len(fw_spans):
                    ffw += load_feats(ffwp, ffw_h, fw_spans[i : i + 1])
                if i < len(bw_spans):
                    fbw += load_feats(fbwp, fbw_h, bw_spans[i : i + 1])
            fw_slot = slot_map(fw_spans)
            bw_slot = slot_map(bw_spans)

            def step(k, psum_pool, wmat, state, ftiles, fslot, out_dram,
                     nsteps, seed, shipped, phase_after=None):
                m = psum_pool.tile([C, BL], mybir.dt.float32)
                rhs = seed if k == 0 else state[:, k * BL : (k + 1) * BL]
                mm = nc.tensor.matmul(
                    m[:], wmat, rhs, start=True, stop=True,
                )
                if phase_after is not None:
                    # pure scheduling edge: pins this chain's phase a fixed
                    # lag behind the other chain so the two never collapse
                    # into the in-phase (serialized, 2x slower) mode
                    add_dep_helper(
                        mm.ins, phase_after.ins, sync=True,
                        reason="cross-chain phase pin",
                    )
                i, j = fslot[k]
                f = ftiles[i]
                tt = nc.vector.tensor_mul(
                    state[:, (k + 1) * BL : (k + 2) * BL],
                    f[:, j * BL : (j + 1) * BL],
                    m[:],
                )
                # ship finished history slots while the scan keeps running;
                # the extra boundary 8 steps before the end keeps the final
                # (unoverlapped) tail transfer tiny
                if (k + 1) % CH == 0 or k + 1 == nsteps or k + 1 == nsteps - 8:
                    lo = shipped["s"] * BL
                    hi = (k + 2) * BL
                    nc.sync.dma_start(
                        out=out_dram[:, lo:hi], in_=state[:, lo:hi]
                    )
                    shipped["s"] = k + 2
                return tt

            # Stagger the backward chain one step behind the forward chain in
            # each engine's (in-order) instruction stream, and pin its phase
            # with an explicit cross-chain edge, so B's ops always slot into
            # the idle gaps of F's latency-bound period instead of collapsing
            # into the in-phase (serialized, 2x slower) mode.
            prev_ttF = None
            shipF, shipB = {"s": 0}, {"s": 0}
            for k in range(NFW):
                ttF = step(k, mpsF, emat, PF, ffw, fw_slot, pf_h, NFW, seedF,
                           shipF)
                if 1 <= k and k - 1 < NBW:
                    step(k - 1, mpsB, ematT, XB, fbw, bw_slot, xb_h, NBW,
                         seedB, shipB, phase_after=prev_ttF)
                prev_ttF = ttF
    nc.compile()
    return nc


def _get_nc():
    global _NC
    if _NC is None:
        _NC = _build_nc()
    return _NC


def _shift_constant(transitions: np.ndarray) -> float:
    """log(Perron eigenvalue of exp(trans)) + E[e^feat] growth correction."""
    tm = transitions.astype(np.float64)
    mx = tm.max()
    Et = np.exp(tm - mx)
    v = np.ones(C) / C
    r = 1.0
    for _ in range(200):
        w = Et.T @ v
        r = np.linalg.norm(w)
        v = w / r
    return float(np.log(r) + mx + 0.5)


def kernel(feats, mask, tags, transitions):
    global LAST_RESULT
    feats = np.asarray(feats, dtype=np.float32)
    mask = np.asarray(mask, dtype=np.int32)
    tags = np.asarray(tags, dtype=np.int32)
    transitions = np.asarray(transitions, dtype=np.float32)

    s = _shift_constant(transitions)
    with np.errstate(under="ignore"):
        emat = np.exp(
            (transitions.astype(np.float64) - s).astype(np.float32)
        ).astype(ml_dtypes.bfloat16)

    konst = np.zeros((C, 2 * C + 2 * BL), dtype=ml_dtypes.bfloat16)
    konst[:, :C] = emat
    konst[:, C : 2 * C] = emat.T
    konst[C - 2, 2 * C : 2 * C + BL] = 1.0        # forward seed: START one-hot
    konst[C - 1, 2 * C + BL : 2 * C + 2 * BL] = 1.0  # backward seed: STOP

    lengths = mask.sum(1)  # [B]

    # forward feats: [B,T,C] -> [C, NFW, B] slices per core
    featsT = np.ascontiguousarray(feats[:, :NFW, :].transpose(2, 1, 0))
    # backward aligned feats: iteration m of sequence b consumes
    # feats[b, len_b-1-m, :]; entries past the sequence start are 0 (exp -> 1)
    fbw_all = np.zeros((B, NBW, C), dtype=np.float32)
    for b in range(B):
        L = int(lengths[b])
        n = min(L, NBW)
        fbw_all[b, :n] = feats[b, L - n : L][::-1]
    fbwT = np.ascontiguousarray(fbw_all.transpose(2, 1, 0))  # [C, NBW, B]

    in_maps = [
        {
            "ffw": np.ascontiguousarray(featsT[:, :, c * BL : (c + 1) * BL]),
            "fbw": np.ascontiguousarray(fbwT[:, :, c * BL : (c + 1) * BL]),
            "konst": konst,
        }
        for c in range(N_CORES)
    ]

    nc = _get_nc()
    res = run_bass_kernel_spmd(nc, in_maps, core_ids=list(range(N_CORES)))
    LAST_RESULT = res

    # ---- unshard / host assembly ----
    logZ = np.zeros(B, dtype=np.float64)
    for c in range(N_CORES):
        pf = np.asarray(res.results[c]["pfout"]).reshape(C, NFW + 1, BL)
        xb = np.asarray(res.results[c]["xbout"]).reshape(C, NBW + 1, BL)
        for b in range(BL):
            bg = c * BL + b
            L = int(lengths[bg])
            t_b = max(0, L - NBW)
            m_b = min(L - 1, NBW - 1)
            num = (
                pf[:, t_b + 1, b].astype(np.float32)
                * xb[:, m_b + 1, b].astype(np.float32)
                / np.exp(feats[bg, t_b, :])
            )
            logZ[bg] = np.log(num.sum(dtype=np.float32)) + (L + 1) * s
    fwd = np.float32(logZ.astype(np.float32).sum())

    # ---- gold-path score (host; pure gather/sum) ----
    r = np.arange(B)
    pad_start = np.concatenate([np.full((B, 1), C - 2, tags.dtype), tags], axis=1)
    pad_stop = np.concatenate([tags, np.full((B, 1), C - 1, tags.dtype)], axis=1)
    pad_stop[r, lengths] = C - 1
    tvals = transitions[pad_start, pad_stop]  # [B,T+1]
    t_score = np.cumsum(tvals, axis=1)[r, lengths].sum(dtype=np.float32)
    fg = np.take_along_axis(feats, tags[:, :, None], axis=2)[..., 0]
    f_score = np.where(mask.astype(bool), fg, np.float32(0.0)).sum(dtype=np.float32)

    nll = (np.float32(fwd) - (t_score + f_score)) / np.float32(B)
    return np.array(nll, dtype=np.float32)

